# revision 24
# baseline (speedup 1.0000x reference)
"""Self-contained Trainium2 Bass kernel for nn_CA_9363028705415 (sparse_attention).

Computes, per batch b:
    Q = relu(x[b] @ qW1 + qb1) @ qW2 + qb2          # [M, K]
    Kt = relu(x[b] @ kW1 + kb1) @ kW2 + kb2         # [M, K]
    S = Q @ Kt.T                                    # [M, M]
    out[b] = softmax(S / rowmax(S), axis=-1)        # max-DIVISION normalization

Shapes: B=16, M=2048, D=128, H=256, K=64.  Output [16, 2048, 2048] f32 (256 MB).

Sharding: data-parallel over batch across 8 NeuronCores; 2 batches/core; tiny
MLP weights replicated.  Single NEFF run SPMD via run_bass_kernel_spmd.

Device writes the output in fp16 (16 MB/core instead of 32 MB); the host
upcasts to f32 after gathering.  fp16 quantization error (~3e-4 rel) is far
below the 2e-2 gate.

x never touches the compute engines: a SWDGE cast-DMA produces a bf16 copy of
each x token-half in DRAM scratch, and a HWDGE xbar transpose-DMA loads
x^T [D, M] straight into SBUF (per-half tiles so the casts/transposes/mlp
pipeline at half granularity).

S is computed in two [128, 1024] PSUM half-tiles (2 banks each; psum_s pool
2 bufs = 4 banks) so the OTHER 4 banks serve a dedicated MLP pool: batch 1's
MLP chunks interleave into batch 0's S loop without stealing S-pipeline slots.

Per 128-row tile:
  PE:  2x2 matmuls -> two [128,1024] f32 PSUM halves
  DVE: per half, fused PSUM->SBUF fp16 copy + running row-max
       (tensor_scalar accum_out=max, 1x mode: fp32 PSUM source);
       reduce_max over the two half-maxes; reciprocal of max ONLY
  ACT: exp(S * (1/max)) over the full fp16 row, fused row-sum accumulate
  DVE: reciprocal of previous tile's sum (separate op so exp never waits on
       the previous accumulator read), then the previous tile's normalize
       multiply at 4x (fp16 in/out SBUF); some norms go to ACT (1x) to
       balance -- NORM_PATTERN
  HWDGE DMA: 1 MB fp16 output chunks (2 row-tiles; final tile split for tail)
"""

import numpy as np
import ml_dtypes

import concourse.bass as bass
import concourse.mybir as mybir
from concourse import bacc
import concourse.tile as tile
from concourse.bass import ts
from concourse.bass_utils import run_bass_kernel_spmd

F32 = mybir.dt.float32
BF16 = mybir.dt.bfloat16
FP16 = mybir.dt.float16
AF = mybir.ActivationFunctionType
ALU = mybir.AluOpType

N_CORES = 8
B, M, D, H, KF = 16, 2048, 128, 256, 64
BPC = B // N_CORES     # batches per core
MT = M // 128          # 16 row-tiles per batch
HM = M // 2            # 1024: half-tile free size

# normalize engine per row-tile: DVE fp16->fp16 runs 4x (~0.6us/tile),
# ACT copy-with-scale is 1x (~2us/tile); ~11/16 on DVE balances the
# engines given ACT also owns the exp.
NORM_PATTERN = (
    "dve", "act", "dve", "dve", "act", "dve", "dve", "act",
    "dve", "dve", "act", "dve", "dve", "act", "dve", "dve",
)


def _evac_bias(nc, engine, out, in_, bias, relu):
    """out = [relu](in_ + bias), bias is [P,1] per-partition AP."""
    if engine == "act":
        nc.scalar.activation(
            out, in_, AF.Relu if relu else AF.Identity, bias=bias, scale=1.0
        )
    else:
        if relu:
            nc.vector.tensor_scalar(out, in_, bias, 0.0, op0=ALU.add, op1=ALU.max)
        else:
            nc.vector.tensor_scalar(out, in_, bias, None, op0=ALU.add)


def _norm(nc, engine, out, t, isum):
    if engine == "act":
        nc.scalar.mul(out, t, isum)
    else:
        nc.vector.tensor_scalar_mul(out, t, isum)


def build_nc():
    nc = bacc.Bacc()

    x = nc.dram_tensor("x", [BPC, M, D], F32, kind="ExternalInput")
    w1d, b1d, w2d, b2d = {}, {}, {}, {}
    for h in ("q", "k"):
        w1d[h] = nc.dram_tensor(f"{h}W1", [D, H], F32, kind="ExternalInput")
        b1d[h] = nc.dram_tensor(f"{h}b1", [H], F32, kind="ExternalInput")
        w2d[h] = nc.dram_tensor(f"{h}W2", [H, KF], F32, kind="ExternalInput")
        b2d[h] = nc.dram_tensor(f"{h}b2", [KF], F32, kind="ExternalInput")
    out = nc.dram_tensor("out", [BPC, M, M], FP16, kind="ExternalOutput")

    ident_np = np.eye(128, dtype=ml_dtypes.bfloat16)
    ident_dram = nc.inline_tensor(ident_np, name="ident_data")

    # [b, p, n, d]: token (n*128+p), feature d
    x_r = x[:].rearrange("b (n p) d -> b p n d", p=128)
    # [b, p, n, m]: out[b, n*128+p, m]
    out_r = out[:].rearrange("b (n p) m -> b p n m", p=128)

    with tile.TileContext(nc) as tc:
        with (
            tc.tile_pool(name="consts", bufs=1) as consts,
            tc.tile_pool(name="xin", bufs=1) as xin_pool,
            tc.tile_pool(name="xt", bufs=1) as xt_pool,
            tc.tile_pool(name="ht", bufs=2) as ht_pool,
            tc.tile_pool(name="qkt", bufs=2) as qkt_pool,
            tc.tile_pool(name="texp", bufs=4) as t_pool,
            tc.tile_pool(name="osb", bufs=6) as out_pool,
            tc.tile_pool(name="small", bufs=8) as small_pool,
            tc.tile_pool(name="psum_s", bufs=2, space="PSUM") as psum_s,
            tc.tile_pool(name="psum_mlp", bufs=2, space="PSUM") as psum_mlp,
        ):
            norm_i = 0

            # ---- weights/biases first (tiny, they gate mlp1), then x
            # f32 halves, all via HWDGE; no SWDGE at all (any gpsimd DMA
            # makes the gpsimd end-of-queue DRAIN block downstream engine
            # ops for its full transfer tail) ----
            xf = {}
            ident = consts.tile([128, 128], BF16, tag="ident")
            nc.sync.dma_start(out=ident, in_=ident_dram[:])
            w1, w2, b1, b2 = {}, {}, {}, {}
            wraw = {}
            for h in ("q", "k"):
                wraw[h, 1] = consts.tile([D, H], F32, tag=f"w1r{h}", name=f"w1r{h}")
                nc.sync.dma_start(out=wraw[h, 1], in_=w1d[h][:])
                wraw[h, 2] = consts.tile(
                    [128, 2, KF], F32, tag=f"w2r{h}", name=f"w2r{h}"
                )
                nc.sync.dma_start(
                    out=wraw[h, 2], in_=w2d[h][:].rearrange("(c p) k -> p c k", p=128)
                )
                b1[h] = consts.tile([128, 2], F32, tag=f"b1{h}", name=f"b1{h}")
                nc.sync.dma_start(
                    out=b1[h], in_=b1d[h][:].rearrange("(c p) -> p c", p=128)
                )
                b2[h] = consts.tile([KF, 1], F32, tag=f"b2{h}", name=f"b2{h}")
                nc.sync.dma_start(
                    out=b2[h], in_=b2d[h][:].rearrange("(k o) -> k o", o=1)
                )
            for half in range(2):
                xf[0, half] = xin_pool.tile(
                    [128, 8, 128], F32, tag=f"xf0{half}", name="xf"
                )
                nc.sync.dma_start(
                    out=xf[0, half], in_=x_r[0][:, half * 8 : (half + 1) * 8, :]
                )
            for half in range(2):
                xf[1, half] = xin_pool.tile(
                    [128, 8, 128], F32, tag=f"xf1{half}", name="xf"
                )
                nc.sync.dma_start(
                    out=xf[1, half], in_=x_r[1][:, half * 8 : (half + 1) * 8, :]
                )
            for h in ("q", "k"):
                w1[h] = consts.tile([D, H], BF16, tag=f"w1{h}", name=f"w1{h}")
                nc.vector.tensor_copy(w1[h], wraw[h, 1])
                w2[h] = consts.tile([128, 2, KF], BF16, tag=f"w2{h}", name=f"w2{h}")
                nc.vector.tensor_copy(w2[h], wraw[h, 2])

            # ---- PE warm-up: dummy matmuls trip the HAM clock gate to
            # 2.4 GHz before the real MLP starts ----
            wu = consts.tile([128, 512], BF16, tag="wu", name="warm")
            nc.vector.memset(wu, 0.0)
            for i in range(8):
                ps_w = psum_s.tile([128, 512], F32, tag="ps", name="ps_warm")
                nc.tensor.matmul(
                    ps_w, lhsT=wu[:, 0:128], rhs=wu, start=True, stop=True
                )

            # ---- xT prep: DVE cast + warm-PE transposes + evac, per half ----
            xT, xsb = {}, {}
            for b in range(BPC):
                for half in range(2):
                    xT[b, half] = xt_pool.tile(
                        [128, HM], BF16, tag=f"xt{b}{half}", name="xT"
                    )
                    xsb[b, half] = xin_pool.tile(
                        [128, 8, 128], BF16, tag=f"xsb{b}{half}", name="xsb"
                    )

            def x_prep(b, half):
                nc.vector.tensor_copy(xsb[b, half], xf[b, half])
                tp = psum_mlp.tile([128, HM], BF16, tag="ps", name="tp")
                for it in range(8):
                    nc.tensor.transpose(
                        tp[:, ts(it, 128)], xsb[b, half][:, it, :], ident
                    )
                for e, fc in (("act", 0), ("dve", 1)):
                    dst = xT[b, half][:, ts(fc, 512)]
                    if e == "act":
                        nc.scalar.copy(dst, tp[:, ts(fc, 512)])
                    else:
                        nc.vector.tensor_copy(dst, tp[:, ts(fc, 512)])

            def phase_a_chunks(b, pools):
                """MLP chunk closures for batch b (fine-grained so they can
                interleave into the previous batch's S loop).  Each chunk uses
                one 2-bank slot from `pools` (round-robin: A0 can use both
                PSUM pools since the S loop isn't running yet; A1 only
                psum_mlp)."""
                pool_i = [0]

                def next_pool():
                    p = pools[pool_i[0] % len(pools)]
                    pool_i[0] += 1
                    return p
                xT_ap = lambda half, fc: xT[b, half][:, ts(fc, 512)]
                # per-(head, pc, half) and per-(head, mh) tiles so the
                # dependency tracking (tile-granular) stays minimal
                ht = {}
                for h in ("q", "k"):
                    for pc in range(2):
                        for half in range(2):
                            ht[h, pc, half] = ht_pool.tile(
                                [128, HM], BF16, tag=f"ht{h}{pc}{half}", name="ht"
                            )
                qkt = {}
                for h in ("q", "k"):
                    for mh in range(2):
                        qkt[h, mh] = qkt_pool.tile(
                            [KF, HM], BF16, tag=f"qkt{h}{mh}", name="qkt"
                        )

                def c_mlp1(h, pc, half):
                    def go():
                        ps1 = next_pool().tile([128, HM], F32, tag="ps", name="ps1")
                        for fc in range(2):
                            nc.tensor.matmul(
                                ps1[:, ts(fc, 512)],
                                lhsT=w1[h][:, ts(pc, 128)],
                                rhs=xT_ap(half, fc),
                                start=True,
                                stop=True,
                            )
                        for e, fc in (("act", 0), ("dve", 1)):
                            _evac_bias(
                                nc,
                                e,
                                ht[h, pc, half][:, ts(fc, 512)],
                                ps1[:, ts(fc, 512)],
                                b1[h][:, pc : pc + 1],
                                relu=True,
                            )
                    return go

                def c_mlp2(h, mh):
                    def go():
                        ps2 = next_pool().tile([KF, HM], F32, tag="ps", name="ps2")
                        for fc in range(2):
                            for kc in range(2):
                                nc.tensor.matmul(
                                    ps2[:, ts(fc, 512)],
                                    lhsT=w2[h][:, kc, :],
                                    rhs=ht[h, kc, mh][:, ts(fc, 512)],
                                    start=(kc == 0),
                                    stop=(kc == 1),
                                )
                        for e, fc in (("act", 0), ("dve", 1)):
                            _evac_bias(
                                nc,
                                e,
                                qkt[h, mh][:, ts(fc, 512)],
                                ps2[:, ts(fc, 512)],
                                b2[h],
                                relu=False,
                            )
                    return go

                chunks = []
                for half in range(2):
                    for h, pc in (("q", 0), ("k", 0), ("q", 1), ("k", 1)):
                        chunks.append(c_mlp1(h, pc, half))
                for mh in range(2):
                    for h in ("q", "k"):
                        chunks.append(c_mlp2(h, mh))
                return qkt, chunks

            def s_loop(b, qkt, next_chunks):
                """S + softmax loop for batch b, interleaving next_chunks
                (next batch's MLP) into the iterations."""
                nonlocal norm_i
                pending = None  # (rt, t_tile, sum_tile)

                def finish(j, t_j, isum_ap):
                    nonlocal norm_i
                    osb = out_pool.tile([128, M], FP16, tag="o", name="osb")
                    _norm(
                        nc,
                        NORM_PATTERN[norm_i % len(NORM_PATTERN)],
                        osb,
                        t_j,
                        isum_ap,
                    )
                    norm_i += 1
                    if j == MT - 1:
                        # tail: split the last tile's DMA in half
                        for hh in range(2):
                            nc.sync.dma_start(
                                out=out_r[b][:, j : j + 1, ts(hh, HM)],
                                in_=osb[:, ts(hh, HM)],
                            )
                    else:
                        nc.sync.dma_start(out=out_r[b][:, j : j + 1, :], in_=osb)

                for rt in range(MT):
                    sc_t = t_pool.tile([128, M], FP16, tag="sc", name="sc")
                    mx2 = small_pool.tile([128, 2], F32, tag="mx", name="mx2")
                    for hf in range(2):
                        ps_s = psum_s.tile([128, HM], F32, tag="ps", name="ps_s")
                        for fc in range(2):
                            nc.tensor.matmul(
                                ps_s[:, ts(fc, 512)],
                                lhsT=qkt["q", rt // 8][:, ts(rt % 8, 128)],
                                rhs=qkt["k", hf][:, ts(fc, 512)],
                                start=True,
                                stop=True,
                            )
                        # fused PSUM->SBUF fp16 evac + row-max of this half
                        nc.vector.tensor_scalar(
                            sc_t[:, ts(hf, HM)],
                            ps_s,
                            0.0,
                            None,
                            op0=ALU.add,
                            op1=ALU.max,
                            accum_out=mx2[:, hf : hf + 1],
                        )
                    mx = small_pool.tile([128, 1], F32, tag="m1", name="mx")
                    nc.vector.reduce_max(mx, mx2, axis=mybir.AxisListType.X)
                    imax = small_pool.tile([128, 1], F32, tag="im", name="imax")
                    nc.vector.reciprocal(imax, mx)

                    sum_t = small_pool.tile([128, 1], F32, tag="sm", name="sum")
                    t_t = t_pool.tile([128, M], FP16, tag="t")
                    nc.scalar.activation(
                        t_t,
                        sc_t,
                        AF.Exp,
                        bias=0.0,
                        scale=imax,
                        accum_out=sum_t,
                    )

                    if pending is not None:
                        pj, pt, psum_t = pending
                        isum = small_pool.tile([128, 1], F32, tag="is", name="isum")
                        nc.vector.reciprocal(isum, psum_t)
                        finish(pj, pt, isum)
                    pending = (rt, t_t, sum_t)

                    if next_chunks:
                        next_chunks.pop(0)()

                pj, pt, psum_t = pending
                isum = small_pool.tile([128, 1], F32, tag="is", name="isum")
                nc.vector.reciprocal(isum, psum_t)
                finish(pj, pt, isum)
                while next_chunks:
                    next_chunks.pop(0)()

            qkt0, chunks0 = phase_a_chunks(0, [psum_mlp, psum_s])
            # A0: prep half 0, its mlp1 chunks, prep half 1, the rest
            x_prep(0, 0)
            for c in chunks0[0:4]:
                c()
            x_prep(0, 1)
            for c in chunks0[4:]:
                c()
            qkt1, chunks1 = phase_a_chunks(1, [psum_mlp])
            chunks1 = (
                [lambda: x_prep(1, 0)]
                + chunks1[0:4]
                + [lambda: x_prep(1, 1)]
                + chunks1[4:]
            )
            s_loop(0, qkt0, chunks1)
            s_loop(1, qkt1, [])
    nc.finalize()
    return nc


_NC_CACHE = None


def _get_nc():
    global _NC_CACHE
    if _NC_CACHE is None:
        _NC_CACHE = build_nc()
    return _NC_CACHE


def run(inputs, trace=False, trace_cores=None):
    """Run on 8 cores; returns (full_output [B,M,M] f32, BassKernelResults)."""
    nc = _get_nc()
    in_maps = []
    x = np.ascontiguousarray(inputs["x"], dtype=np.float32)
    for c in range(N_CORES):
        im = {"x": np.ascontiguousarray(x[c * BPC : (c + 1) * BPC])}
        for k in ("qW1", "qb1", "qW2", "qb2", "kW1", "kb1", "kW2", "kb2"):
            im[k] = np.ascontiguousarray(inputs[k], dtype=np.float32)
        in_maps.append(im)
    res = run_bass_kernel_spmd(
        nc,
        in_maps,
        core_ids=list(range(N_CORES)),
        trace=trace,
        trace_cores=trace_cores,
    )
    outs = [np.asarray(r["out"]) for r in res.results]
    full = np.concatenate(outs, axis=0).astype(np.float32)
    assert full.shape == (B, M, M) and full.dtype == np.float32
    return full, res


def kernel(**inputs) -> np.ndarray:
    out, _ = run(inputs, trace=False)
    return out


# revision 25
# speedup vs baseline: 1.0121x; 1.0121x over previous
"""Self-contained Trainium2 Bass kernel for nn_CA_9363028705415 (sparse_attention).

Computes, per batch b:
    Q = relu(x[b] @ qW1 + qb1) @ qW2 + qb2          # [M, K]
    Kt = relu(x[b] @ kW1 + kb1) @ kW2 + kb2         # [M, K]
    S = Q @ Kt.T                                    # [M, M]
    out[b] = softmax(S / rowmax(S), axis=-1)        # max-DIVISION normalization

Shapes: B=16, M=2048, D=128, H=256, K=64.  Output [16, 2048, 2048] f32 (256 MB).

Sharding: data-parallel over batch across 8 NeuronCores; 2 batches/core; tiny
MLP weights replicated.  Single NEFF run SPMD via run_bass_kernel_spmd.

Device writes the output in fp16 (16 MB/core instead of 32 MB); the host
upcasts to f32 after gathering.  fp16 quantization error (~3e-4 rel) is far
below the 2e-2 gate.

x never touches the compute engines: a SWDGE cast-DMA produces a bf16 copy of
each x token-half in DRAM scratch, and a HWDGE xbar transpose-DMA loads
x^T [D, M] straight into SBUF (per-half tiles so the casts/transposes/mlp
pipeline at half granularity).

S is computed in two [128, 1024] PSUM half-tiles (2 banks each; psum_s pool
2 bufs = 4 banks) so the OTHER 4 banks serve a dedicated MLP pool: batch 1's
MLP chunks interleave into batch 0's S loop without stealing S-pipeline slots.

Per 128-row tile:
  PE:  2x2 matmuls -> two [128,1024] f32 PSUM halves
  DVE: per half, fused PSUM->SBUF fp16 copy + running row-max
       (tensor_scalar accum_out=max, 1x mode: fp32 PSUM source);
       reduce_max over the two half-maxes; reciprocal of max ONLY
  ACT: exp(S * (1/max)) over the full fp16 row, fused row-sum accumulate
  DVE: reciprocal of previous tile's sum (separate op so exp never waits on
       the previous accumulator read), then the previous tile's normalize
       multiply at 4x (fp16 in/out SBUF); some norms go to ACT (1x) to
       balance -- NORM_PATTERN
  HWDGE DMA: 1 MB fp16 output chunks (2 row-tiles; final tile split for tail)
"""

import numpy as np
import ml_dtypes

import concourse.bass as bass
import concourse.mybir as mybir
from concourse import bacc
import concourse.tile as tile
from concourse.bass import ts
from concourse.bass_utils import run_bass_kernel_spmd

F32 = mybir.dt.float32
BF16 = mybir.dt.bfloat16
FP16 = mybir.dt.float16
AF = mybir.ActivationFunctionType
ALU = mybir.AluOpType

N_CORES = 8
B, M, D, H, KF = 16, 2048, 128, 256, 64
BPC = B // N_CORES     # batches per core
MT = M // 128          # 16 row-tiles per batch
HM = M // 2            # 1024: half-tile free size

# normalize engine per row-tile: DVE fp16->fp16 runs 4x (~0.6us/tile),
# ACT copy-with-scale is 1x (~2us/tile); ~11/16 on DVE balances the
# engines given ACT also owns the exp.
NORM_PATTERN = (
    "dve", "act", "dve", "dve", "act", "dve", "dve", "act",
    "dve", "dve", "act", "dve", "dve", "act", "dve", "dve",
)


def _evac_bias(nc, engine, out, in_, bias, relu):
    """out = [relu](in_ + bias), bias is [P,1] per-partition AP."""
    if engine == "act":
        nc.scalar.activation(
            out, in_, AF.Relu if relu else AF.Identity, bias=bias, scale=1.0
        )
    else:
        if relu:
            nc.vector.tensor_scalar(out, in_, bias, 0.0, op0=ALU.add, op1=ALU.max)
        else:
            nc.vector.tensor_scalar(out, in_, bias, None, op0=ALU.add)


def _norm(nc, engine, out, t, isum):
    if engine == "act":
        nc.scalar.mul(out, t, isum)
    else:
        nc.vector.tensor_scalar_mul(out, t, isum)


def build_nc():
    nc = bacc.Bacc()

    x = nc.dram_tensor("x", [BPC, M, D], F32, kind="ExternalInput")
    w1d, b1d, w2d, b2d = {}, {}, {}, {}
    for h in ("q", "k"):
        w1d[h] = nc.dram_tensor(f"{h}W1", [D, H], F32, kind="ExternalInput")
        b1d[h] = nc.dram_tensor(f"{h}b1", [H], F32, kind="ExternalInput")
        w2d[h] = nc.dram_tensor(f"{h}W2", [H, KF], F32, kind="ExternalInput")
        b2d[h] = nc.dram_tensor(f"{h}b2", [KF], F32, kind="ExternalInput")
    out = nc.dram_tensor("out", [BPC, M, M], FP16, kind="ExternalOutput")

    ident_np = np.eye(128, dtype=ml_dtypes.bfloat16)
    ident_dram = nc.inline_tensor(ident_np, name="ident_data")

    # [b, p, n, d]: token (n*128+p), feature d
    x_r = x[:].rearrange("b (n p) d -> b p n d", p=128)
    # [b, p, n, m]: out[b, n*128+p, m]
    out_r = out[:].rearrange("b (n p) m -> b p n m", p=128)

    with tile.TileContext(nc) as tc:
        with (
            tc.tile_pool(name="consts", bufs=1) as consts,
            tc.tile_pool(name="xin", bufs=1) as xin_pool,
            tc.tile_pool(name="xt", bufs=1) as xt_pool,
            tc.tile_pool(name="ht", bufs=2) as ht_pool,
            tc.tile_pool(name="qkt", bufs=2) as qkt_pool,
            tc.tile_pool(name="texp", bufs=6) as t_pool,
            tc.tile_pool(name="osb", bufs=8) as out_pool,
            tc.tile_pool(name="small", bufs=8) as small_pool,
            tc.tile_pool(name="psum_s", bufs=2, space="PSUM") as psum_s,
            tc.tile_pool(name="psum_mlp", bufs=2, space="PSUM") as psum_mlp,
        ):
            norm_i = 0

            # ---- weights/biases first (tiny, they gate mlp1), then x
            # f32 halves, all via HWDGE; no SWDGE at all (any gpsimd DMA
            # makes the gpsimd end-of-queue DRAIN block downstream engine
            # ops for its full transfer tail) ----
            # x0 heads the sync queue; weights/biases issue in parallel on
            # the scalar HWDGE queue (each DIRECT2D issue costs ~0.7us and
            # serializes per-queue -- splitting queues halves the ramp)
            xf = {}
            for half in range(2):
                xf[0, half] = xin_pool.tile(
                    [128, 8, 128], F32, tag=f"xf0{half}", name="xf"
                )
                nc.sync.dma_start(
                    out=xf[0, half], in_=x_r[0][:, half * 8 : (half + 1) * 8, :]
                )
            ident = consts.tile([128, 128], BF16, tag="ident")
            nc.sync.dma_start(out=ident, in_=ident_dram[:])
            for half in range(2):
                xf[1, half] = xin_pool.tile(
                    [128, 8, 128], F32, tag=f"xf1{half}", name="xf"
                )
                nc.sync.dma_start(
                    out=xf[1, half], in_=x_r[1][:, half * 8 : (half + 1) * 8, :]
                )
            w1, w2, b1, b2 = {}, {}, {}, {}
            wraw = {}
            for h in ("q", "k"):
                wraw[h, 1] = consts.tile([D, H], F32, tag=f"w1r{h}", name=f"w1r{h}")
                nc.scalar.dma_start(out=wraw[h, 1], in_=w1d[h][:])
                wraw[h, 2] = consts.tile(
                    [128, 2, KF], F32, tag=f"w2r{h}", name=f"w2r{h}"
                )
                nc.scalar.dma_start(
                    out=wraw[h, 2], in_=w2d[h][:].rearrange("(c p) k -> p c k", p=128)
                )
                b1[h] = consts.tile([128, 2], F32, tag=f"b1{h}", name=f"b1{h}")
                nc.scalar.dma_start(
                    out=b1[h], in_=b1d[h][:].rearrange("(c p) -> p c", p=128)
                )
                b2[h] = consts.tile([KF, 1], F32, tag=f"b2{h}", name=f"b2{h}")
                nc.scalar.dma_start(
                    out=b2[h], in_=b2d[h][:].rearrange("(k o) -> k o", o=1)
                )
            for h in ("q", "k"):
                w1[h] = consts.tile([D, H], BF16, tag=f"w1{h}", name=f"w1{h}")
                nc.vector.tensor_copy(w1[h], wraw[h, 1])
                w2[h] = consts.tile([128, 2, KF], BF16, tag=f"w2{h}", name=f"w2{h}")
                nc.vector.tensor_copy(w2[h], wraw[h, 2])

            # ---- PE warm-up: dummy matmuls trip the HAM clock gate to
            # 2.4 GHz before the real MLP starts ----
            wu = consts.tile([128, 512], BF16, tag="wu", name="warm")
            nc.vector.memset(wu, 0.0)
            for i in range(8):
                ps_w = psum_s.tile([128, 512], F32, tag="ps", name="ps_warm")
                nc.tensor.matmul(
                    ps_w, lhsT=wu[:, 0:128], rhs=wu, start=True, stop=True
                )

            # ---- xT prep: DVE cast + warm-PE transposes + evac, per half ----
            xT, xsb = {}, {}
            for b in range(BPC):
                for half in range(2):
                    xT[b, half] = xt_pool.tile(
                        [128, HM], BF16, tag=f"xt{b}{half}", name="xT"
                    )
                    xsb[b, half] = xin_pool.tile(
                        [128, 8, 128], BF16, tag=f"xsb{b}{half}", name="xsb"
                    )

            def x_prep(b, half):
                nc.vector.tensor_copy(xsb[b, half], xf[b, half])
                tp = psum_mlp.tile([128, HM], BF16, tag="ps", name="tp")
                for it in range(8):
                    nc.tensor.transpose(
                        tp[:, ts(it, 128)], xsb[b, half][:, it, :], ident
                    )
                for e, fc in (("act", 0), ("dve", 1)):
                    dst = xT[b, half][:, ts(fc, 512)]
                    if e == "act":
                        nc.scalar.copy(dst, tp[:, ts(fc, 512)])
                    else:
                        nc.vector.tensor_copy(dst, tp[:, ts(fc, 512)])

            def phase_a_chunks(b, pools):
                """MLP chunk closures for batch b (fine-grained so they can
                interleave into the previous batch's S loop).  Each chunk uses
                one 2-bank slot from `pools` (round-robin: A0 can use both
                PSUM pools since the S loop isn't running yet; A1 only
                psum_mlp)."""
                pool_i = [0]

                def next_pool():
                    p = pools[pool_i[0] % len(pools)]
                    pool_i[0] += 1
                    return p
                xT_ap = lambda half, fc: xT[b, half][:, ts(fc, 512)]
                # per-(head, pc, half) and per-(head, mh) tiles so the
                # dependency tracking (tile-granular) stays minimal
                ht = {}
                for h in ("q", "k"):
                    for pc in range(2):
                        for half in range(2):
                            ht[h, pc, half] = ht_pool.tile(
                                [128, HM], BF16, tag=f"ht{h}{pc}{half}", name="ht"
                            )
                qkt = {}
                for h in ("q", "k"):
                    for mh in range(2):
                        qkt[h, mh] = qkt_pool.tile(
                            [KF, HM], BF16, tag=f"qkt{h}{mh}", name="qkt"
                        )

                def c_mlp1(h, pc, half):
                    def go():
                        ps1 = next_pool().tile([128, HM], F32, tag="ps", name="ps1")
                        for fc in range(2):
                            nc.tensor.matmul(
                                ps1[:, ts(fc, 512)],
                                lhsT=w1[h][:, ts(pc, 128)],
                                rhs=xT_ap(half, fc),
                                start=True,
                                stop=True,
                            )
                        for e, fc in (("act", 0), ("dve", 1)):
                            _evac_bias(
                                nc,
                                e,
                                ht[h, pc, half][:, ts(fc, 512)],
                                ps1[:, ts(fc, 512)],
                                b1[h][:, pc : pc + 1],
                                relu=True,
                            )
                    return go

                def c_mlp2(h, mh):
                    def go():
                        ps2 = next_pool().tile([KF, HM], F32, tag="ps", name="ps2")
                        for fc in range(2):
                            for kc in range(2):
                                nc.tensor.matmul(
                                    ps2[:, ts(fc, 512)],
                                    lhsT=w2[h][:, kc, :],
                                    rhs=ht[h, kc, mh][:, ts(fc, 512)],
                                    start=(kc == 0),
                                    stop=(kc == 1),
                                )
                        for e, fc in (("act", 0), ("dve", 1)):
                            _evac_bias(
                                nc,
                                e,
                                qkt[h, mh][:, ts(fc, 512)],
                                ps2[:, ts(fc, 512)],
                                b2[h],
                                relu=False,
                            )
                    return go

                chunks = []
                for half in range(2):
                    for h, pc in (("q", 0), ("k", 0), ("q", 1), ("k", 1)):
                        chunks.append(c_mlp1(h, pc, half))
                for mh in range(2):
                    for h in ("q", "k"):
                        chunks.append(c_mlp2(h, mh))
                return qkt, chunks

            def s_loop(b, qkt, next_chunks):
                """S + softmax loop for batch b, interleaving next_chunks
                (next batch's MLP) into the iterations."""
                nonlocal norm_i
                pending = None  # (rt, t_tile, sum_tile)

                def finish(j, t_j, isum_ap):
                    nonlocal norm_i
                    osb = out_pool.tile([128, M], FP16, tag="o", name="osb")
                    _norm(
                        nc,
                        NORM_PATTERN[norm_i % len(NORM_PATTERN)],
                        osb,
                        t_j,
                        isum_ap,
                    )
                    norm_i += 1
                    if j == MT - 1:
                        # tail: split the last tile's DMA in half
                        for hh in range(2):
                            nc.sync.dma_start(
                                out=out_r[b][:, j : j + 1, ts(hh, HM)],
                                in_=osb[:, ts(hh, HM)],
                            )
                    else:
                        nc.sync.dma_start(out=out_r[b][:, j : j + 1, :], in_=osb)

                for rt in range(MT):
                    sc_t = t_pool.tile([128, M], FP16, tag="sc", name="sc")
                    mx2 = small_pool.tile([128, 2], F32, tag="mx", name="mx2")
                    for hf in range(2):
                        ps_s = psum_s.tile([128, HM], F32, tag="ps", name="ps_s")
                        for fc in range(2):
                            nc.tensor.matmul(
                                ps_s[:, ts(fc, 512)],
                                lhsT=qkt["q", rt // 8][:, ts(rt % 8, 128)],
                                rhs=qkt["k", hf][:, ts(fc, 512)],
                                start=True,
                                stop=True,
                            )
                        # fused PSUM->SBUF fp16 evac + row-max of this half
                        nc.vector.tensor_scalar(
                            sc_t[:, ts(hf, HM)],
                            ps_s,
                            0.0,
                            None,
                            op0=ALU.add,
                            op1=ALU.max,
                            accum_out=mx2[:, hf : hf + 1],
                        )
                    mx = small_pool.tile([128, 1], F32, tag="m1", name="mx")
                    nc.vector.reduce_max(mx, mx2, axis=mybir.AxisListType.X)
                    imax = small_pool.tile([128, 1], F32, tag="im", name="imax")
                    nc.vector.reciprocal(imax, mx)

                    sum_t = small_pool.tile([128, 1], F32, tag="sm", name="sum")
                    t_t = t_pool.tile([128, M], FP16, tag="t")
                    nc.scalar.activation(
                        t_t,
                        sc_t,
                        AF.Exp,
                        bias=0.0,
                        scale=imax,
                        accum_out=sum_t,
                    )

                    if pending is not None:
                        pj, pt, psum_t = pending
                        isum = small_pool.tile([128, 1], F32, tag="is", name="isum")
                        nc.vector.reciprocal(isum, psum_t)
                        finish(pj, pt, isum)
                    pending = (rt, t_t, sum_t)

                    if next_chunks:
                        next_chunks.pop(0)()

                pj, pt, psum_t = pending
                isum = small_pool.tile([128, 1], F32, tag="is", name="isum")
                nc.vector.reciprocal(isum, psum_t)
                finish(pj, pt, isum)
                while next_chunks:
                    next_chunks.pop(0)()

            qkt0, chunks0 = phase_a_chunks(0, [psum_mlp, psum_s])
            # A0: prep half 0, its mlp1 chunks, prep half 1, the rest
            x_prep(0, 0)
            for c in chunks0[0:4]:
                c()
            x_prep(0, 1)
            for c in chunks0[4:]:
                c()
            qkt1, chunks1 = phase_a_chunks(1, [psum_mlp])
            chunks1 = (
                [lambda: x_prep(1, 0)]
                + chunks1[0:4]
                + [lambda: x_prep(1, 1)]
                + chunks1[4:]
            )
            s_loop(0, qkt0, chunks1)
            s_loop(1, qkt1, [])
    nc.finalize()
    return nc


_NC_CACHE = None


def _get_nc():
    global _NC_CACHE
    if _NC_CACHE is None:
        _NC_CACHE = build_nc()
    return _NC_CACHE


def run(inputs, trace=False, trace_cores=None):
    """Run on 8 cores; returns (full_output [B,M,M] f32, BassKernelResults)."""
    nc = _get_nc()
    in_maps = []
    x = np.ascontiguousarray(inputs["x"], dtype=np.float32)
    for c in range(N_CORES):
        im = {"x": np.ascontiguousarray(x[c * BPC : (c + 1) * BPC])}
        for k in ("qW1", "qb1", "qW2", "qb2", "kW1", "kb1", "kW2", "kb2"):
            im[k] = np.ascontiguousarray(inputs[k], dtype=np.float32)
        in_maps.append(im)
    res = run_bass_kernel_spmd(
        nc,
        in_maps,
        core_ids=list(range(N_CORES)),
        trace=trace,
        trace_cores=trace_cores,
    )
    outs = [np.asarray(r["out"]) for r in res.results]
    full = np.concatenate(outs, axis=0).astype(np.float32)
    assert full.shape == (B, M, M) and full.dtype == np.float32
    return full, res


def kernel(**inputs) -> np.ndarray:
    out, _ = run(inputs, trace=False)
    return out


# revision 27
# speedup vs baseline: 1.0217x; 1.0095x over previous
"""Self-contained Trainium2 Bass kernel for nn_CA_9363028705415 (sparse_attention).

Computes, per batch b:
    Q = relu(x[b] @ qW1 + qb1) @ qW2 + qb2          # [M, K]
    Kt = relu(x[b] @ kW1 + kb1) @ kW2 + kb2         # [M, K]
    S = Q @ Kt.T                                    # [M, M]
    out[b] = softmax(S / rowmax(S), axis=-1)        # max-DIVISION normalization

Shapes: B=16, M=2048, D=128, H=256, K=64.  Output [16, 2048, 2048] f32 (256 MB).

Sharding: data-parallel over batch across 8 NeuronCores; 2 batches/core; tiny
MLP weights replicated.  Single NEFF run SPMD via run_bass_kernel_spmd.

Device writes the output in fp16 (16 MB/core instead of 32 MB); the host
upcasts to f32 after gathering.  fp16 quantization error (~3e-4 rel) is far
below the 2e-2 gate.

x never touches the compute engines: a SWDGE cast-DMA produces a bf16 copy of
each x token-half in DRAM scratch, and a HWDGE xbar transpose-DMA loads
x^T [D, M] straight into SBUF (per-half tiles so the casts/transposes/mlp
pipeline at half granularity).

S is computed in two [128, 1024] PSUM half-tiles (2 banks each; psum_s pool
2 bufs = 4 banks) so the OTHER 4 banks serve a dedicated MLP pool: batch 1's
MLP chunks interleave into batch 0's S loop without stealing S-pipeline slots.

Per 128-row tile:
  PE:  2x2 matmuls -> two [128,1024] f32 PSUM halves
  DVE: per half, fused PSUM->SBUF fp16 copy + running row-max
       (tensor_scalar accum_out=max, 1x mode: fp32 PSUM source);
       reduce_max over the two half-maxes; reciprocal of max ONLY
  ACT: exp(S * (1/max)) over the full fp16 row, fused row-sum accumulate
  DVE: reciprocal of previous tile's sum (separate op so exp never waits on
       the previous accumulator read), then the previous tile's normalize
       multiply at 4x (fp16 in/out SBUF); some norms go to ACT (1x) to
       balance -- NORM_PATTERN
  HWDGE DMA: 1 MB fp16 output chunks (2 row-tiles; final tile split for tail)
"""

import numpy as np
import ml_dtypes

import concourse.bass as bass
import concourse.mybir as mybir
from concourse import bacc
import concourse.tile as tile
from concourse.bass import ts
from concourse.bass_utils import run_bass_kernel_spmd

F32 = mybir.dt.float32
BF16 = mybir.dt.bfloat16
FP16 = mybir.dt.float16
AF = mybir.ActivationFunctionType
ALU = mybir.AluOpType

N_CORES = 8
B, M, D, H, KF = 16, 2048, 128, 256, 64
BPC = B // N_CORES     # batches per core
MT = M // 128          # 16 row-tiles per batch
HM = M // 2            # 1024: half-tile free size

# normalize engine per row-tile: DVE fp16->fp16 runs 4x (~0.6us/tile),
# ACT copy-with-scale is 1x (~2us/tile); ~11/16 on DVE balances the
# engines given ACT also owns the exp.
NORM_PATTERN = ("dve", "dve", "act") * 10 + ("dve", "act")


def _evac_bias(nc, engine, out, in_, bias, relu):
    """out = [relu](in_ + bias), bias is [P,1] per-partition AP."""
    if engine == "act":
        nc.scalar.activation(
            out, in_, AF.Relu if relu else AF.Identity, bias=bias, scale=1.0
        )
    else:
        if relu:
            nc.vector.tensor_scalar(out, in_, bias, 0.0, op0=ALU.add, op1=ALU.max)
        else:
            nc.vector.tensor_scalar(out, in_, bias, None, op0=ALU.add)


def _norm(nc, engine, out, t, isum):
    if engine == "act":
        nc.scalar.mul(out, t, isum)
    else:
        nc.vector.tensor_scalar_mul(out, t, isum)


def build_nc():
    nc = bacc.Bacc()

    x = nc.dram_tensor("x", [BPC, M, D], F32, kind="ExternalInput")
    w1d, b1d, w2d, b2d = {}, {}, {}, {}
    for h in ("q", "k"):
        w1d[h] = nc.dram_tensor(f"{h}W1", [D, H], F32, kind="ExternalInput")
        b1d[h] = nc.dram_tensor(f"{h}b1", [H], F32, kind="ExternalInput")
        w2d[h] = nc.dram_tensor(f"{h}W2", [H, KF], F32, kind="ExternalInput")
        b2d[h] = nc.dram_tensor(f"{h}b2", [KF], F32, kind="ExternalInput")
    out = nc.dram_tensor("out", [BPC, M, M], FP16, kind="ExternalOutput")

    ident_np = np.eye(128, dtype=ml_dtypes.bfloat16)
    ident_dram = nc.inline_tensor(ident_np, name="ident_data")

    # [b, p, n, d]: token (n*128+p), feature d
    x_r = x[:].rearrange("b (n p) d -> b p n d", p=128)
    # [b, p, n, m]: out[b, n*128+p, m]
    out_r = out[:].rearrange("b (n p) m -> b p n m", p=128)

    with tile.TileContext(nc) as tc:
        with (
            tc.tile_pool(name="consts", bufs=1) as consts,
            tc.tile_pool(name="xin", bufs=1) as xin_pool,
            tc.tile_pool(name="xt", bufs=1) as xt_pool,
            tc.tile_pool(name="ht", bufs=2) as ht_pool,
            tc.tile_pool(name="qkt", bufs=2) as qkt_pool,
            tc.tile_pool(name="texp", bufs=6) as t_pool,
            tc.tile_pool(name="osb", bufs=8) as out_pool,
            tc.tile_pool(name="small", bufs=8) as small_pool,
            tc.tile_pool(name="psum_s", bufs=2, space="PSUM") as psum_s,
            tc.tile_pool(name="psum_mlp", bufs=2, space="PSUM") as psum_mlp,
        ):
            norm_i = 0

            # ---- weights/biases first (tiny, they gate mlp1), then x
            # f32 halves, all via HWDGE; no SWDGE at all (any gpsimd DMA
            # makes the gpsimd end-of-queue DRAIN block downstream engine
            # ops for its full transfer tail) ----
            # x0 heads the sync queue; weights/biases issue in parallel on
            # the scalar HWDGE queue (each DIRECT2D issue costs ~0.7us and
            # serializes per-queue -- splitting queues halves the ramp)
            xf = {}
            for half in range(2):
                xf[0, half] = xin_pool.tile(
                    [128, 8, 128], F32, tag=f"xf0{half}", name="xf"
                )
                nc.sync.dma_start(
                    out=xf[0, half], in_=x_r[0][:, half * 8 : (half + 1) * 8, :]
                )
            ident = consts.tile([128, 128], BF16, tag="ident")
            nc.sync.dma_start(out=ident, in_=ident_dram[:])
            for half in range(2):
                xf[1, half] = xin_pool.tile(
                    [128, 8, 128], F32, tag=f"xf1{half}", name="xf"
                )
                nc.sync.dma_start(
                    out=xf[1, half], in_=x_r[1][:, half * 8 : (half + 1) * 8, :]
                )
            w1, w2, b1, b2 = {}, {}, {}, {}
            wraw = {}
            for h in ("q", "k"):
                wraw[h, 1] = consts.tile([D, H], F32, tag=f"w1r{h}", name=f"w1r{h}")
                nc.scalar.dma_start(out=wraw[h, 1], in_=w1d[h][:])
                wraw[h, 2] = consts.tile(
                    [128, 2, KF], F32, tag=f"w2r{h}", name=f"w2r{h}"
                )
                nc.scalar.dma_start(
                    out=wraw[h, 2], in_=w2d[h][:].rearrange("(c p) k -> p c k", p=128)
                )
                b1[h] = consts.tile([128, 2], F32, tag=f"b1{h}", name=f"b1{h}")
                nc.scalar.dma_start(
                    out=b1[h], in_=b1d[h][:].rearrange("(c p) -> p c", p=128)
                )
                b2[h] = consts.tile([KF, 1], F32, tag=f"b2{h}", name=f"b2{h}")
                nc.scalar.dma_start(
                    out=b2[h], in_=b2d[h][:].rearrange("(k o) -> k o", o=1)
                )
            for h in ("q", "k"):
                w1[h] = consts.tile([D, H], BF16, tag=f"w1{h}", name=f"w1{h}")
                nc.vector.tensor_copy(w1[h], wraw[h, 1])
                w2[h] = consts.tile([128, 2, KF], BF16, tag=f"w2{h}", name=f"w2{h}")
                nc.vector.tensor_copy(w2[h], wraw[h, 2])

            # ---- PE warm-up: dummy matmuls trip the HAM clock gate to
            # 2.4 GHz before the real MLP starts ----
            wu = consts.tile([128, 512], BF16, tag="wu", name="warm")
            nc.vector.memset(wu, 0.0)
            for i in range(8):
                ps_w = psum_s.tile([128, 512], F32, tag="ps", name="ps_warm")
                nc.tensor.matmul(
                    ps_w, lhsT=wu[:, 0:128], rhs=wu, start=True, stop=True
                )

            # ---- xT prep: DVE cast + warm-PE transposes + evac, per half ----
            xT, xsb = {}, {}
            for b in range(BPC):
                for half in range(2):
                    xT[b, half] = xt_pool.tile(
                        [128, HM], BF16, tag=f"xt{b}{half}", name="xT"
                    )
                    xsb[b, half] = xin_pool.tile(
                        [128, 8, 128], BF16, tag=f"xsb{b}{half}", name="xsb"
                    )

            def x_prep(b, half):
                nc.vector.tensor_copy(xsb[b, half], xf[b, half])
                tp = psum_mlp.tile([128, HM], BF16, tag="ps", name="tp")
                for it in range(8):
                    nc.tensor.transpose(
                        tp[:, ts(it, 128)], xsb[b, half][:, it, :], ident
                    )
                for e, fc in (("act", 0), ("dve", 1)):
                    dst = xT[b, half][:, ts(fc, 512)]
                    if e == "act":
                        nc.scalar.copy(dst, tp[:, ts(fc, 512)])
                    else:
                        nc.vector.tensor_copy(dst, tp[:, ts(fc, 512)])

            def phase_a_chunks(b, pools):
                """MLP chunk closures for batch b (fine-grained so they can
                interleave into the previous batch's S loop).  Each chunk uses
                one 2-bank slot from `pools` (round-robin: A0 can use both
                PSUM pools since the S loop isn't running yet; A1 only
                psum_mlp)."""
                pool_i = [0]

                def next_pool():
                    p = pools[pool_i[0] % len(pools)]
                    pool_i[0] += 1
                    return p
                # per-(head, pc, half) and per-(head, mh) tiles so the
                # dependency tracking (tile-granular) stays minimal
                ht = {}
                for h in ("q", "k"):
                    for pc in range(2):
                        for half in range(2):
                            ht[h, pc, half] = ht_pool.tile(
                                [128, HM], BF16, tag=f"ht{h}{pc}{half}", name="ht"
                            )
                qkt = {}
                for h in ("q", "k"):
                    for mh in range(2):
                        qkt[h, mh] = qkt_pool.tile(
                            [KF, HM], BF16, tag=f"qkt{h}{mh}", name="qkt"
                        )

                def c_mlp1(h, pc, half):
                    def go():
                        ps1 = next_pool().tile([128, HM], F32, tag="ps", name="ps1")
                        for fc in range(2):
                            nc.tensor.matmul(
                                ps1[:, ts(fc, 512)],
                                lhsT=w1[h][:, ts(pc, 128)],
                                rhs=xT[b, half][:, ts(fc, 512)],
                                start=True,
                                stop=True,
                            )
                        for e, fc in (("act", 0), ("dve", 1)):
                            _evac_bias(
                                nc,
                                e,
                                ht[h, pc, half][:, ts(fc, 512)],
                                ps1[:, ts(fc, 512)],
                                b1[h][:, pc : pc + 1],
                                relu=True,
                            )
                    return go

                def c_mlp2(h, mh):
                    def go():
                        ps2 = next_pool().tile([KF, HM], F32, tag="ps", name="ps2")
                        for fc in range(2):
                            for kc in range(2):
                                nc.tensor.matmul(
                                    ps2[:, ts(fc, 512)],
                                    lhsT=w2[h][:, kc, :],
                                    rhs=ht[h, kc, mh][:, ts(fc, 512)],
                                    start=(kc == 0),
                                    stop=(kc == 1),
                                )
                        for e, fc in (("act", 0), ("dve", 1)):
                            _evac_bias(
                                nc,
                                e,
                                qkt[h, mh][:, ts(fc, 512)],
                                ps2[:, ts(fc, 512)],
                                b2[h],
                                relu=False,
                            )
                    return go

                chunks = []
                for half in range(2):
                    for h, pc in (("q", 0), ("k", 0), ("q", 1), ("k", 1)):
                        chunks.append(c_mlp1(h, pc, half))
                for mh in range(2):
                    for h in ("q", "k"):
                        chunks.append(c_mlp2(h, mh))
                return qkt, chunks

            def s_loop(b, qkt, next_chunks):
                """S + softmax loop for batch b, interleaving next_chunks
                (next batch's MLP) into the iterations."""
                nonlocal norm_i
                pending = None  # (rt, t_tile, sum_tile)

                def finish(j, t_j, isum_ap):
                    nonlocal norm_i
                    osb = out_pool.tile([128, M], FP16, tag="o", name="osb")
                    _norm(
                        nc,
                        NORM_PATTERN[norm_i % len(NORM_PATTERN)],
                        osb,
                        t_j,
                        isum_ap,
                    )
                    norm_i += 1
                    if j == MT - 1:
                        # tail: split the last tile's DMA in half
                        for hh in range(2):
                            nc.sync.dma_start(
                                out=out_r[b][:, j : j + 1, ts(hh, HM)],
                                in_=osb[:, ts(hh, HM)],
                            )
                    else:
                        nc.sync.dma_start(out=out_r[b][:, j : j + 1, :], in_=osb)

                for rt in range(MT):
                    sc_t = t_pool.tile([128, M], FP16, tag="sc", name="sc")
                    mx2 = small_pool.tile([128, 2], F32, tag="mx", name="mx2")
                    for hf in range(2):
                        ps_s = psum_s.tile([128, HM], F32, tag="ps", name="ps_s")
                        for fc in range(2):
                            nc.tensor.matmul(
                                ps_s[:, ts(fc, 512)],
                                lhsT=qkt["q", rt // 8][:, ts(rt % 8, 128)],
                                rhs=qkt["k", hf][:, ts(fc, 512)],
                                start=True,
                                stop=True,
                            )
                        # fused PSUM->SBUF fp16 evac + row-max of this half
                        nc.vector.tensor_scalar(
                            sc_t[:, ts(hf, HM)],
                            ps_s,
                            0.0,
                            None,
                            op0=ALU.add,
                            op1=ALU.max,
                            accum_out=mx2[:, hf : hf + 1],
                        )
                    mx = small_pool.tile([128, 1], F32, tag="m1", name="mx")
                    nc.vector.reduce_max(mx, mx2, axis=mybir.AxisListType.X)
                    imax = small_pool.tile([128, 1], F32, tag="im", name="imax")
                    nc.vector.reciprocal(imax, mx)

                    sum_t = small_pool.tile([128, 1], F32, tag="sm", name="sum")
                    t_t = t_pool.tile([128, M], FP16, tag="t")
                    nc.scalar.activation(
                        t_t,
                        sc_t,
                        AF.Exp,
                        bias=0.0,
                        scale=imax,
                        accum_out=sum_t,
                    )

                    if pending is not None:
                        pj, pt, psum_t = pending
                        isum = small_pool.tile([128, 1], F32, tag="is", name="isum")
                        nc.vector.reciprocal(isum, psum_t)
                        finish(pj, pt, isum)
                    pending = (rt, t_t, sum_t)

                    if next_chunks:
                        next_chunks.pop(0)()

                pj, pt, psum_t = pending
                isum = small_pool.tile([128, 1], F32, tag="is", name="isum")
                nc.vector.reciprocal(isum, psum_t)
                finish(pj, pt, isum)
                while next_chunks:
                    next_chunks.pop(0)()

            qkt0, chunks0 = phase_a_chunks(0, [psum_mlp, psum_s])
            # A0: prep half 0, its mlp1 chunks, prep half 1, the rest
            x_prep(0, 0)
            for c in chunks0[0:4]:
                c()
            x_prep(0, 1)
            for c in chunks0[4:]:
                c()
            qkt1, chunks1 = phase_a_chunks(1, [psum_mlp])
            chunks1 = (
                [lambda: x_prep(1, 0)]
                + chunks1[0:4]
                + [lambda: x_prep(1, 1)]
                + chunks1[4:]
            )
            s_loop(0, qkt0, chunks1)
            s_loop(1, qkt1, [])
    nc.finalize()
    return nc


_NC_CACHE = None


def _get_nc():
    global _NC_CACHE
    if _NC_CACHE is None:
        _NC_CACHE = build_nc()
    return _NC_CACHE


def run(inputs, trace=False, trace_cores=None):
    """Run on 8 cores; returns (full_output [B,M,M] f32, BassKernelResults)."""
    nc = _get_nc()
    in_maps = []
    x = np.ascontiguousarray(inputs["x"], dtype=np.float32)
    for c in range(N_CORES):
        im = {"x": np.ascontiguousarray(x[c * BPC : (c + 1) * BPC])}
        for k in ("qW1", "qb1", "qW2", "qb2", "kW1", "kb1", "kW2", "kb2"):
            im[k] = np.ascontiguousarray(inputs[k], dtype=np.float32)
        in_maps.append(im)
    res = run_bass_kernel_spmd(
        nc,
        in_maps,
        core_ids=list(range(N_CORES)),
        trace=trace,
        trace_cores=trace_cores,
    )
    outs = [np.asarray(r["out"]) for r in res.results]
    full = np.concatenate(outs, axis=0).astype(np.float32)
    assert full.shape == (B, M, M) and full.dtype == np.float32
    return full, res


def kernel(**inputs) -> np.ndarray:
    out, _ = run(inputs, trace=False)
    return out


# revision 30
# speedup vs baseline: 1.0245x; 1.0027x over previous
"""Self-contained Trainium2 Bass kernel for nn_CA_9363028705415 (sparse_attention).

Computes, per batch b:
    Q = relu(x[b] @ qW1 + qb1) @ qW2 + qb2          # [M, K]
    Kt = relu(x[b] @ kW1 + kb1) @ kW2 + kb2         # [M, K]
    S = Q @ Kt.T                                    # [M, M]
    out[b] = softmax(S / rowmax(S), axis=-1)        # max-DIVISION normalization

Shapes: B=16, M=2048, D=128, H=256, K=64.  Output [16, 2048, 2048] f32 (256 MB).

Sharding: data-parallel over batch across 8 NeuronCores; 2 batches/core; tiny
MLP weights replicated.  Single NEFF run SPMD via run_bass_kernel_spmd.

Device writes the output in fp16 (16 MB/core instead of 32 MB); the host
upcasts to f32 after gathering.  fp16 quantization error (~3e-4 rel) is far
below the 2e-2 gate.

x never touches the compute engines: a SWDGE cast-DMA produces a bf16 copy of
each x token-half in DRAM scratch, and a HWDGE xbar transpose-DMA loads
x^T [D, M] straight into SBUF (per-half tiles so the casts/transposes/mlp
pipeline at half granularity).

S is computed in two [128, 1024] PSUM half-tiles (2 banks each; psum_s pool
2 bufs = 4 banks) so the OTHER 4 banks serve a dedicated MLP pool: batch 1's
MLP chunks interleave into batch 0's S loop without stealing S-pipeline slots.

Per 128-row tile:
  PE:  2x2 matmuls -> two [128,1024] f32 PSUM halves
  DVE: per half, fused PSUM->SBUF fp16 copy + running row-max
       (tensor_scalar accum_out=max, 1x mode: fp32 PSUM source);
       reduce_max over the two half-maxes; reciprocal of max ONLY
  ACT: exp(S * (1/max)) over the full fp16 row, fused row-sum accumulate
  DVE: reciprocal of previous tile's sum (separate op so exp never waits on
       the previous accumulator read), then the previous tile's normalize
       multiply at 4x (fp16 in/out SBUF); some norms go to ACT (1x) to
       balance -- NORM_PATTERN
  HWDGE DMA: 1 MB fp16 output chunks (2 row-tiles; final tile split for tail)
"""

import numpy as np
import ml_dtypes

import concourse.bass as bass
import concourse.mybir as mybir
from concourse import bacc
import concourse.tile as tile
from concourse.bass import ts
from concourse.bass_utils import run_bass_kernel_spmd

F32 = mybir.dt.float32
BF16 = mybir.dt.bfloat16
FP16 = mybir.dt.float16
AF = mybir.ActivationFunctionType
ALU = mybir.AluOpType

N_CORES = 8
B, M, D, H, KF = 16, 2048, 128, 256, 64
BPC = B // N_CORES     # batches per core
MT = M // 128          # 16 row-tiles per batch
HM = M // 2            # 1024: half-tile free size

# normalize engine per row-tile: DVE fp16->fp16 runs 4x (~0.6us/tile),
# ACT copy-with-scale is 1x (~2us/tile); ~11/16 on DVE balances the
# engines given ACT also owns the exp.
NORM_PATTERN = ("dve", "dve", "act") * 10 + ("dve", "act")


def _evac_bias(nc, engine, out, in_, bias, relu):
    """out = [relu](in_ + bias), bias is [P,1] per-partition AP."""
    if engine == "act":
        nc.scalar.activation(
            out, in_, AF.Relu if relu else AF.Identity, bias=bias, scale=1.0
        )
    else:
        if relu:
            nc.vector.tensor_scalar(out, in_, bias, 0.0, op0=ALU.add, op1=ALU.max)
        else:
            nc.vector.tensor_scalar(out, in_, bias, None, op0=ALU.add)


def _norm(nc, engine, out, t, isum):
    if engine == "act":
        nc.scalar.mul(out, t, isum)
    else:
        nc.vector.tensor_scalar_mul(out, t, isum)


def build_nc():
    nc = bacc.Bacc()

    x = nc.dram_tensor("x", [BPC, M, D], F32, kind="ExternalInput")
    w1d, b1d, w2d, b2d = {}, {}, {}, {}
    for h in ("q", "k"):
        w1d[h] = nc.dram_tensor(f"{h}W1", [D, H], F32, kind="ExternalInput")
        b1d[h] = nc.dram_tensor(f"{h}b1", [H], F32, kind="ExternalInput")
        w2d[h] = nc.dram_tensor(f"{h}W2", [H, KF], F32, kind="ExternalInput")
        b2d[h] = nc.dram_tensor(f"{h}b2", [KF], F32, kind="ExternalInput")
    out = nc.dram_tensor("out", [BPC, M, M], FP16, kind="ExternalOutput")

    ident_np = np.eye(128, dtype=ml_dtypes.bfloat16)
    ident_dram = nc.inline_tensor(ident_np, name="ident_data")

    # [b, p, n, d]: token (n*128+p), feature d
    x_r = x[:].rearrange("b (n p) d -> b p n d", p=128)
    # [b, p, n, m]: out[b, n*128+p, m]
    out_r = out[:].rearrange("b (n p) m -> b p n m", p=128)

    with tile.TileContext(nc) as tc:
        with (
            tc.tile_pool(name="consts", bufs=1) as consts,
            tc.tile_pool(name="xin", bufs=1) as xin_pool,
            tc.tile_pool(name="xt", bufs=1) as xt_pool,
            tc.tile_pool(name="ht", bufs=2) as ht_pool,
            tc.tile_pool(name="qkt", bufs=2) as qkt_pool,
            tc.tile_pool(name="texp", bufs=6) as t_pool,
            tc.tile_pool(name="osb", bufs=8) as out_pool,
            tc.tile_pool(name="small", bufs=8) as small_pool,
            tc.tile_pool(name="psum_s", bufs=2, space="PSUM") as psum_s,
            tc.tile_pool(name="psum_mlp", bufs=2, space="PSUM") as psum_mlp,
        ):
            norm_i = 0

            # ---- weights/biases first (tiny, they gate mlp1), then x
            # f32 halves, all via HWDGE; no SWDGE at all (any gpsimd DMA
            # makes the gpsimd end-of-queue DRAIN block downstream engine
            # ops for its full transfer tail) ----
            # x0 heads the sync queue; weights/biases issue in parallel on
            # the scalar HWDGE queue (each DIRECT2D issue costs ~0.7us and
            # serializes per-queue -- splitting queues halves the ramp)
            xf = {}
            for half in range(2):
                xf[0, half] = xin_pool.tile(
                    [128, 8, 128], F32, tag=f"xf0{half}", name="xf"
                )
                nc.sync.dma_start(
                    out=xf[0, half], in_=x_r[0][:, half * 8 : (half + 1) * 8, :]
                )
            ident = consts.tile([128, 128], BF16, tag="ident")
            nc.scalar.dma_start(out=ident, in_=ident_dram[:])
            for half in range(2):
                xf[1, half] = xin_pool.tile(
                    [128, 8, 128], F32, tag=f"xf1{half}", name="xf"
                )
                nc.sync.dma_start(
                    out=xf[1, half], in_=x_r[1][:, half * 8 : (half + 1) * 8, :]
                )
            w1, w2, b1, b2 = {}, {}, {}, {}
            wraw = {}
            for h in ("q", "k"):
                wraw[h, 1] = consts.tile([D, H], F32, tag=f"w1r{h}", name=f"w1r{h}")
                nc.scalar.dma_start(out=wraw[h, 1], in_=w1d[h][:])
                wraw[h, 2] = consts.tile(
                    [128, 2, KF], F32, tag=f"w2r{h}", name=f"w2r{h}"
                )
                nc.scalar.dma_start(
                    out=wraw[h, 2], in_=w2d[h][:].rearrange("(c p) k -> p c k", p=128)
                )
                b1[h] = consts.tile([128, 2], F32, tag=f"b1{h}", name=f"b1{h}")
                nc.scalar.dma_start(
                    out=b1[h], in_=b1d[h][:].rearrange("(c p) -> p c", p=128)
                )
                b2[h] = consts.tile([KF, 1], F32, tag=f"b2{h}", name=f"b2{h}")
                nc.scalar.dma_start(
                    out=b2[h], in_=b2d[h][:].rearrange("(k o) -> k o", o=1)
                )
            for h in ("q", "k"):
                w1[h] = consts.tile([D, H], BF16, tag=f"w1{h}", name=f"w1{h}")
                w2[h] = consts.tile([128, 2, KF], BF16, tag=f"w2{h}", name=f"w2{h}")

            # ---- PE warm-up: dummy matmuls trip the HAM clock gate to
            # 2.4 GHz before the real MLP starts ----
            wu = consts.tile([128, 512], BF16, tag="wu", name="warm")
            nc.vector.memset(wu, 0.0)
            for i in range(8):
                ps_w = psum_s.tile([128, 512], F32, tag="ps", name="ps_warm")
                nc.tensor.matmul(
                    ps_w, lhsT=wu[:, 0:128], rhs=wu, start=True, stop=True
                )

            # ---- xT prep: DVE cast + warm-PE transposes + evac, per half ----
            xT, xsb = {}, {}
            for b in range(BPC):
                for half in range(2):
                    xT[b, half] = xt_pool.tile(
                        [128, HM], BF16, tag=f"xt{b}{half}", name="xT"
                    )
                    xsb[b, half] = xin_pool.tile(
                        [128, 8, 128], BF16, tag=f"xsb{b}{half}", name="xsb"
                    )

            def x_prep(b, half):
                nc.vector.tensor_copy(xsb[b, half], xf[b, half])
                tp = psum_mlp.tile([128, HM], BF16, tag="ps", name="tp")
                for it in range(8):
                    nc.tensor.transpose(
                        tp[:, ts(it, 128)], xsb[b, half][:, it, :], ident
                    )
                for e, fc in (("act", 0), ("dve", 1)):
                    dst = xT[b, half][:, ts(fc, 512)]
                    if e == "act":
                        nc.scalar.copy(dst, tp[:, ts(fc, 512)])
                    else:
                        nc.vector.tensor_copy(dst, tp[:, ts(fc, 512)])

            def phase_a_chunks(b, pools):
                """MLP chunk closures for batch b (fine-grained so they can
                interleave into the previous batch's S loop).  Each chunk uses
                one 2-bank slot from `pools` (round-robin: A0 can use both
                PSUM pools since the S loop isn't running yet; A1 only
                psum_mlp)."""
                pool_i = [0]

                def next_pool():
                    p = pools[pool_i[0] % len(pools)]
                    pool_i[0] += 1
                    return p
                # per-(head, pc, half) and per-(head, mh) tiles so the
                # dependency tracking (tile-granular) stays minimal
                ht = {}
                for h in ("q", "k"):
                    for pc in range(2):
                        for half in range(2):
                            ht[h, pc, half] = ht_pool.tile(
                                [128, HM], BF16, tag=f"ht{h}{pc}{half}", name="ht"
                            )
                qkt = {}
                for h in ("q", "k"):
                    for mh in range(2):
                        qkt[h, mh] = qkt_pool.tile(
                            [KF, HM], BF16, tag=f"qkt{h}{mh}", name="qkt"
                        )

                def c_mlp1(h, pc, half):
                    def go():
                        ps1 = next_pool().tile([128, HM], F32, tag="ps", name="ps1")
                        for fc in range(2):
                            nc.tensor.matmul(
                                ps1[:, ts(fc, 512)],
                                lhsT=w1[h][:, ts(pc, 128)],
                                rhs=xT[b, half][:, ts(fc, 512)],
                                start=True,
                                stop=True,
                            )
                        for e, fc in (("act", 0), ("dve", 1)):
                            _evac_bias(
                                nc,
                                e,
                                ht[h, pc, half][:, ts(fc, 512)],
                                ps1[:, ts(fc, 512)],
                                b1[h][:, pc : pc + 1],
                                relu=True,
                            )
                    return go

                def c_mlp2(h, mh):
                    def go():
                        ps2 = next_pool().tile([KF, HM], F32, tag="ps", name="ps2")
                        for fc in range(2):
                            for kc in range(2):
                                nc.tensor.matmul(
                                    ps2[:, ts(fc, 512)],
                                    lhsT=w2[h][:, kc, :],
                                    rhs=ht[h, kc, mh][:, ts(fc, 512)],
                                    start=(kc == 0),
                                    stop=(kc == 1),
                                )
                        for e, fc in (("act", 0), ("dve", 1)):
                            _evac_bias(
                                nc,
                                e,
                                qkt[h, mh][:, ts(fc, 512)],
                                ps2[:, ts(fc, 512)],
                                b2[h],
                                relu=False,
                            )
                    return go

                chunks = []
                for half in range(2):
                    for h, pc in (("q", 0), ("k", 0), ("q", 1), ("k", 1)):
                        chunks.append(c_mlp1(h, pc, half))
                for mh in range(2):
                    for h in ("q", "k"):
                        chunks.append(c_mlp2(h, mh))
                return qkt, chunks

            def s_loop(b, qkt, next_chunks):
                """S + softmax loop for batch b, interleaving next_chunks
                (next batch's MLP) into the iterations."""
                nonlocal norm_i
                pending = None  # (rt, t_tile, sum_tile)

                def finish(j, t_j, isum_ap):
                    nonlocal norm_i
                    osb = out_pool.tile([128, M], FP16, tag="o", name="osb")
                    _norm(
                        nc,
                        NORM_PATTERN[norm_i % len(NORM_PATTERN)],
                        osb,
                        t_j,
                        isum_ap,
                    )
                    norm_i += 1
                    if j == MT - 1:
                        # tail: split the last tile's DMA in half
                        for hh in range(2):
                            nc.sync.dma_start(
                                out=out_r[b][:, j : j + 1, ts(hh, HM)],
                                in_=osb[:, ts(hh, HM)],
                            )
                    else:
                        nc.sync.dma_start(out=out_r[b][:, j : j + 1, :], in_=osb)

                for rt in range(MT):
                    sc_t = t_pool.tile([128, M], FP16, tag="sc", name="sc")
                    mx2 = small_pool.tile([128, 2], F32, tag="mx", name="mx2")
                    for hf in range(2):
                        ps_s = psum_s.tile([128, HM], F32, tag="ps", name="ps_s")
                        for fc in range(2):
                            nc.tensor.matmul(
                                ps_s[:, ts(fc, 512)],
                                lhsT=qkt["q", rt // 8][:, ts(rt % 8, 128)],
                                rhs=qkt["k", hf][:, ts(fc, 512)],
                                start=True,
                                stop=True,
                            )
                        # fused PSUM->SBUF fp16 evac + row-max of this half
                        nc.vector.tensor_scalar(
                            sc_t[:, ts(hf, HM)],
                            ps_s,
                            0.0,
                            None,
                            op0=ALU.add,
                            op1=ALU.max,
                            accum_out=mx2[:, hf : hf + 1],
                        )
                    mx = small_pool.tile([128, 1], F32, tag="m1", name="mx")
                    nc.vector.reduce_max(mx, mx2, axis=mybir.AxisListType.X)
                    imax = small_pool.tile([128, 1], F32, tag="im", name="imax")
                    nc.vector.reciprocal(imax, mx)

                    sum_t = small_pool.tile([128, 1], F32, tag="sm", name="sum")
                    t_t = t_pool.tile([128, M], FP16, tag="t")
                    nc.scalar.activation(
                        t_t,
                        sc_t,
                        AF.Exp,
                        bias=0.0,
                        scale=imax,
                        accum_out=sum_t,
                    )

                    if pending is not None:
                        pj, pt, psum_t = pending
                        isum = small_pool.tile([128, 1], F32, tag="is", name="isum")
                        nc.vector.reciprocal(isum, psum_t)
                        finish(pj, pt, isum)
                    pending = (rt, t_t, sum_t)

                    if next_chunks:
                        next_chunks.pop(0)()

                pj, pt, psum_t = pending
                isum = small_pool.tile([128, 1], F32, tag="is", name="isum")
                nc.vector.reciprocal(isum, psum_t)
                finish(pj, pt, isum)
                while next_chunks:
                    next_chunks.pop(0)()

            qkt0, chunks0 = phase_a_chunks(0, [psum_mlp, psum_s])
            # A0: prep half 0, its mlp1 chunks, prep half 1, the rest.
            # The DVE weight casts sit AFTER the first x chain so the xsb
            # cast isn't stuck behind them waiting for the weight loads.
            x_prep(0, 0)
            for h in ("q", "k"):
                nc.vector.tensor_copy(w1[h], wraw[h, 1])
                nc.vector.tensor_copy(w2[h], wraw[h, 2])
            for c in chunks0[0:4]:
                c()
            x_prep(0, 1)
            for c in chunks0[4:]:
                c()
            qkt1, chunks1 = phase_a_chunks(1, [psum_mlp])
            chunks1 = (
                [lambda: x_prep(1, 0)]
                + chunks1[0:4]
                + [lambda: x_prep(1, 1)]
                + chunks1[4:]
            )
            s_loop(0, qkt0, chunks1)
            s_loop(1, qkt1, [])
    nc.finalize()
    return nc


_NC_CACHE = None


def _get_nc():
    global _NC_CACHE
    if _NC_CACHE is None:
        _NC_CACHE = build_nc()
    return _NC_CACHE


def run(inputs, trace=False, trace_cores=None):
    """Run on 8 cores; returns (full_output [B,M,M] f32, BassKernelResults)."""
    nc = _get_nc()
    in_maps = []
    x = np.ascontiguousarray(inputs["x"], dtype=np.float32)
    for c in range(N_CORES):
        im = {"x": np.ascontiguousarray(x[c * BPC : (c + 1) * BPC])}
        for k in ("qW1", "qb1", "qW2", "qb2", "kW1", "kb1", "kW2", "kb2"):
            im[k] = np.ascontiguousarray(inputs[k], dtype=np.float32)
        in_maps.append(im)
    res = run_bass_kernel_spmd(
        nc,
        in_maps,
        core_ids=list(range(N_CORES)),
        trace=trace,
        trace_cores=trace_cores,
    )
    outs = [np.asarray(r["out"]) for r in res.results]
    full = np.concatenate(outs, axis=0).astype(np.float32)
    assert full.shape == (B, M, M) and full.dtype == np.float32
    return full, res


def kernel(**inputs) -> np.ndarray:
    out, _ = run(inputs, trace=False)
    return out
